# revision 1
# baseline (speedup 1.0000x reference)
"""2-layer GCN on 8 Trainium2 NeuronCores (Bass/Tile), self-contained.

Sharding: nodes partitioned across 8 cores (12500 rows each). Per core:
  table1 = (x @ W1) * dinv  for its shard  -> AllGather -> full gather table
  L1 aggregation of its dst rows via dma_gather + one-hot segment matmuls
  table2 = relu(agg * dinv + b1) * dinv    -> AllGather
  L2 aggregation (64-dim), then @ W2 + b2, sigmoid.

Math identity used: norm factorizes (dinv[src]*dinv[dst]) and aggregation
commutes with the linear layer, so gathers always move 64-float rows.
"""
import math

import numpy as np

N = 100000
E_RAW = 1600000
DIN = 64
DH = 64
DOUT = 16
NCORE = 8
SH = 12500            # nodes per core
T = 98                # dst tiles per core (128 nodes each)
SHP = 128 * T         # padded shard rows = 12544
NTAB = NCORE * SHP    # gather table rows = 100352
RSIZE = 32768         # int16 index window (rows)
NRANGE = 4            # ceil(NTAB / RSIZE)
SB = 2                # tiles per gather call-group (small groups ->
                      # ~1280-idx calls, fine-grained msg-buffer release)
PADP = 999.0          # dst_local marker for padded edges (kills one-hot col)
MAX_CALL_IDX = 1280   # cap idxs per dma_gather: 81 descs/engine-ring,
                      # ~3 calls resident per queue ring -> keeps all 4
                      # SWDGE queues draining concurrently

_CACHE = {}


def _host_prep(edge_index):
    """Build per-core gather/one-hot metadata. Returns (meta, percore)."""
    src = np.concatenate([edge_index[0], np.arange(N, dtype=np.int64)])
    dst = np.concatenate([edge_index[1], np.arange(N, dtype=np.int64)])
    src = src.astype(np.int64)
    dst = dst.astype(np.int64)
    E = src.shape[0]

    k = dst // SH                      # owning core of each edge (by dst)
    j = dst % SH
    p_dst = j // T                     # psum partition of dst
    t_dst = j % T                      # dst tile
    trow = (src // SH) * SHP + (src % SH)   # gather-table row of src
    r = trow // RSIZE
    loc = trow % RSIZE

    # fake self-edges for pad rows j in [SH, SHP) so psum slots get written
    pj = np.arange(SH, SHP, dtype=np.int64)
    n_pad_nodes = pj.shape[0]
    fk = np.repeat(np.arange(NCORE, dtype=np.int64), n_pad_nodes)
    fj = np.tile(pj, NCORE)
    ftrow = fk * SHP + fj
    k = np.concatenate([k, fk])
    p_dst = np.concatenate([p_dst, fj // T])
    t_dst = np.concatenate([t_dst, fj % T])
    r = np.concatenate([r, ftrow // RSIZE])
    loc = np.concatenate([loc, ftrow % RSIZE])

    gid = ((k * T + t_dst) * NRANGE + r).astype(np.int64)
    order = np.argsort(gid, kind="stable")
    gid_s = gid[order]
    loc_s = loc[order].astype(np.int32)
    p_s = p_dst[order].astype(np.float32)

    ngroups = NCORE * T * NRANGE
    counts = np.bincount(gid_s, minlength=ngroups).reshape(NCORE, T, NRANGE)
    starts = np.zeros(ngroups + 1, np.int64)
    np.cumsum(counts.reshape(-1), out=starts[1:])

    # uniform chunk structure across cores
    n_chunk = np.maximum(1, -(-counts.max(axis=0) // 128))      # [T, NRANGE]
    # gather call groups: for each sb, range: cols = sum of n_chunk over tiles
    sbs = [list(range(s, min(s + SB, T))) for s in range(0, T, SB)]

    # degree per (core, j) includes self-loop; pads get deg 1 via fake edges
    degc = np.bincount((dst // SH) * SHP + (dst % SH), minlength=NCORE * SHP)
    degc = degc.reshape(NCORE, SHP).astype(np.int64)
    degc[:, SH:] = 1

    meta = dict(n_chunk=n_chunk, sbs=sbs)
    tot_chunks = int(n_chunk.sum())
    meta["tot_chunks"] = tot_chunks

    percore = []
    for kk in range(NCORE):
        # idx arrays per range (in sb-major, then tile order), dl columns in
        # (sb, r, t, chunk) order matching the kernel's consumption order.
        idx_r = [[] for _ in range(NRANGE)]
        dl_cols = []

        def group_arrays(tt, rr):
            g = (kk * T + tt) * NRANGE + rr
            cnt = counts[kk, tt, rr]
            npad = n_chunk[tt, rr] * 128
            lo = np.zeros(npad, np.int32)
            pp = np.full(npad, PADP, np.float32)
            lo[:cnt] = loc_s[starts[g]:starts[g] + cnt]
            pp[:cnt] = p_s[starts[g]:starts[g] + cnt]
            return lo, pp

        for tiles in sbs:
            # idx arrays: (sb, range, tile) order — matches gather calls
            for rr in range(NRANGE):
                for tt in tiles:
                    idx_r[rr].append(group_arrays(tt, rr)[0])
            # dl columns: (sb, tile, range) order — matches chunk consumption
            for tt in tiles:
                for rr in range(NRANGE):
                    pp = group_arrays(tt, rr)[1]
                    dl_cols.append(pp.reshape(-1, 128).T)  # [128, nchunk]
        idx_arrs = []
        for rr in range(NRANGE):
            flat = np.concatenate(idx_r[rr]).astype(np.int16)
            wrapped = flat.reshape(-1, 16).T                 # [16, n/16]
            idx_arrs.append(np.tile(wrapped, (8, 1)).copy()) # [128, n/16]
        dl = np.concatenate(dl_cols, axis=1).astype(np.float32)  # [128, totch]
        dln = -dl

        # rowptr-style deg inputs: rs/re with re-rs = deg, layout [128, T]
        d = degc[kk].astype(np.float32)                      # [SHP]
        rs = np.zeros(SHP, np.float32)
        np.cumsum(d[:-1], out=rs[1:])
        re = rs + d
        rs_pm = rs.reshape(128, T)
        re_pm = re.reshape(128, T)
        percore.append(dict(idx=idx_arrs, dl=dl, dln=dln, rs=rs_pm, re=re_pm))
    return meta, percore


def _build_nc(meta, npass=1, msg_bufs=6, oh_bufs=6, mode="full", max_call=MAX_CALL_IDX, ix_bufs=3, act_mod=5, ps_bufs=4):
    import concourse.bacc as bacc
    import concourse.mybir as mybir
    from concourse.masks import make_identity
    from concourse.tile import TileContext

    f32 = mybir.dt.float32
    n_chunk = meta["n_chunk"]
    sbs = meta["sbs"]
    tot_chunks = meta["tot_chunks"]
    idx_lens = meta["idx_lens"]

    nc = bacc.Bacc("TRN2", target_bir_lowering=False, debug=False,
                   num_devices=NCORE, num_swdge_queues=4)
    x_pm = nc.dram_tensor("x_pm", [128, T * DIN], f32, kind="ExternalInput")
    rs_d = nc.dram_tensor("rs", [128, T], f32, kind="ExternalInput")
    re_d = nc.dram_tensor("re", [128, T], f32, kind="ExternalInput")
    w1_d = nc.dram_tensor("w1", [DIN, DH], f32, kind="ExternalInput")
    w2_d = nc.dram_tensor("w2", [DH, DOUT], f32, kind="ExternalInput")
    b1_d = nc.dram_tensor("b1b", [128, DH], f32, kind="ExternalInput")
    b2_d = nc.dram_tensor("b2b", [128, DOUT], f32, kind="ExternalInput")
    io_d = nc.dram_tensor("iota2d", [128, 128], f32, kind="ExternalInput")
    dl_d = nc.dram_tensor("dl", [128, tot_chunks], f32, kind="ExternalInput")
    dln_d = nc.dram_tensor("dln", [128, tot_chunks], f32, kind="ExternalInput")
    ix_d = [nc.dram_tensor(f"ix{rr}", [128, idx_lens[rr] // 16],
                           mybir.dt.int16, kind="ExternalInput")
            for rr in range(NRANGE)]
    y_d = nc.dram_tensor("y_pm", [128, T * DOUT], f32, kind="ExternalOutput")

    qrot = [0]

    def nextq():
        qrot[0] = (qrot[0] + 1) % 4
        return qrot[0]

    with TileContext(nc) as tc:
        with (
            tc.tile_pool(name="const", bufs=1) as constp,
            tc.tile_pool(name="big", bufs=1) as bigp,
            tc.tile_pool(name="msg", bufs=msg_bufs) as msgp,
            tc.tile_pool(name="ixp", bufs=ix_bufs) as ixp,
            tc.tile_pool(name="ohp", bufs=oh_bufs) as ohp,
            tc.tile_pool(name="work", bufs=3) as workp,
            tc.tile_pool(name="ps", bufs=ps_bufs, space="PSUM") as psp,
            tc.tile_pool(name="ps2", bufs=2, space="PSUM") as ps2p,
            tc.tile_pool(name="dram", bufs=1, space="DRAM") as dramp,
        ):
            ident = constp.tile([128, 128], f32)
            make_identity(nc, ident[:])
            w1_s = constp.tile([DIN, DH], f32)
            nc.sync.dma_start(out=w1_s[:], in_=w1_d[:])
            w2_s = constp.tile([DH, DOUT], f32)
            nc.sync.dma_start(out=w2_s[:], in_=w2_d[:])
            b1_s = constp.tile([128, DH], f32)
            nc.sync.dma_start(out=b1_s[:], in_=b1_d[:])
            b2_s = constp.tile([128, DOUT], f32)
            nc.sync.dma_start(out=b2_s[:], in_=b2_d[:])
            iota_s = constp.tile([128, 128], f32)
            nc.sync.dma_start(out=iota_s[:], in_=io_d[:])
            dl_s = constp.tile([128, tot_chunks], f32)
            nc.sync.dma_start(out=dl_s[:], in_=dl_d[:])
            dln_s = constp.tile([128, tot_chunks], f32)
            nc.sync.dma_start(out=dln_s[:], in_=dln_d[:])

            # deg -> dinv
            rs_s = constp.tile([128, T], f32)
            nc.sync.dma_start(out=rs_s[:], in_=rs_d[:])
            re_s = constp.tile([128, T], f32)
            nc.sync.dma_start(out=re_s[:], in_=re_d[:])
            deg_s = constp.tile([128, T], f32)
            nc.vector.tensor_tensor(out=deg_s[:], in0=re_s[:], in1=rs_s[:],
                                    op=mybir.AluOpType.subtract)
            sq_s = constp.tile([128, T], f32)
            nc.scalar.activation(sq_s[:], deg_s[:],
                                 mybir.ActivationFunctionType.Sqrt)
            dinv = constp.tile([128, T], f32)
            nc.vector.reciprocal(dinv[:], sq_s[:])

            def one_pass():
                # ---- table1 = (x @ W1) * dinv ----
                xbuf = bigp.tile([128, T * DIN], f32, tag="xbuf")
                nc.sync.dma_start(out=xbuf[:], in_=x_pm[:])
                tab1 = bigp.tile([128, T * DH], f32, tag="tab1")
                for t in range(T):
                    xt_ps = ps2p.tile([DIN, 128], f32, tag="tr")
                    nc.tensor.transpose(out=xt_ps[:],
                                        in_=xbuf[:, t * DIN:(t + 1) * DIN],
                                        identity=ident[:])
                    xt_s = workp.tile([DIN, 128], f32, tag="xt")
                    nc.vector.tensor_copy(out=xt_s[:], in_=xt_ps[:])
                    h_ps = psp.tile([128, DH], f32, tag="agg", name="h_ps")
                    nc.tensor.matmul(h_ps[:], lhsT=xt_s[:], rhs=w1_s[:],
                                     start=True, stop=True)
                    nc.vector.tensor_scalar_mul(tab1[:, t * DH:(t + 1) * DH],
                                                h_ps[:], dinv[:, t:t + 1])

                ag1_in = dramp.tile([SHP, DH], f32)
                nc.sync.dma_start(
                    out=ag1_in[:].rearrange("(p t) f -> p (t f)", p=128),
                    in_=tab1[:])
                tab1_full = dramp.tile([NTAB, DH], f32, addr_space="Shared")
                nc.gpsimd.collective_compute(
                    "AllGather", mybir.AluOpType.bypass,
                    replica_groups=[list(range(NCORE))],
                    ins=[ag1_in[:]], outs=[tab1_full[:]])

                def aggregate(table_full, epilogue):
                    """Gather+segment-matmul over all tiles; epilogue(t, psum)."""
                    ch_col = 0          # global chunk column (dl index)
                    ix_off = [0] * NRANGE
                    for tiles in sbs:
                        msgs = {}
                        for rr in range(NRANGE):
                            cols = int(sum(n_chunk[tt, rr] for tt in tiles))
                            if mode == "none":
                                msgs[rr] = None
                                continue
                            m = msgp.tile([128, cols, DH], f32, tag=f"m{rr}")
                            nidx = cols * 128
                            ixt = ixp.tile([128, nidx // 16], mybir.dt.int16,
                                           tag=f"ix{rr}")
                            nc.sync.dma_start(
                                out=ixt[:],
                                in_=ix_d[rr][:, ix_off[rr]:ix_off[rr] + nidx // 16])
                            ix_off[rr] += nidx // 16
                            rlen = min(RSIZE, NTAB - rr * RSIZE)
                            # split into ring-safe calls at chunk granularity
                            c0 = 0
                            while c0 < cols:
                                cn = min(cols - c0, max_call // 128)
                                nc.gpsimd.dma_gather(
                                    out_ap=m[:, c0:c0 + cn, :],
                                    in_ap=table_full[rr * RSIZE:rr * RSIZE + rlen, :],
                                    idxs_ap=ixt[:, c0 * 8:(c0 + cn) * 8],
                                    num_idxs=cn * 128,
                                    num_idxs_reg=cn * 128,
                                    elem_size=DH,
                                    queue_num=nextq(),
                                    single_packet=False,
                                )
                                c0 += cn
                            msgs[rr] = m
                        # consume: per tile, chunks from each range buffer
                        roff = {rr: 0 for rr in range(NRANGE)}
                        for ti, tt in enumerate(tiles):
                            if mode == "full":
                                ps = psp.tile([128, DH], f32, tag="agg",
                                              name="ps_agg")
                            else:
                                ps = None
                            nch = int(sum(n_chunk[tt, rr] for rr in range(NRANGE)))
                            done = 0
                            for rr in range(NRANGE):
                                base = int(sum(n_chunk[t2, rr] for t2 in tiles[:ti]))
                                for c in range(int(n_chunk[tt, rr])):
                                    if mode not in ("gather", "none"):
                                        oh = ohp.tile([128, 128], f32, tag="oh")
                                        if act_mod and ch_col % act_mod == 0:
                                            sq = ohp.tile([128, 128], f32,
                                                          tag="sq")
                                            nc.scalar.activation(
                                                sq[:], iota_s[:],
                                                mybir.ActivationFunctionType.Square,
                                                bias=dln_s[:, ch_col:ch_col + 1])
                                            nc.scalar.activation(
                                                oh[:], sq[:],
                                                mybir.ActivationFunctionType.Relu,
                                                bias=1.0, scale=-1.0)
                                        else:
                                            nc.vector.tensor_scalar(
                                                out=oh[:], in0=iota_s[:],
                                                scalar1=dl_s[:, ch_col:ch_col + 1],
                                                scalar2=None,
                                                op0=mybir.AluOpType.is_equal)
                                        if mode == "full":
                                            nc.tensor.matmul(
                                                ps[:], lhsT=oh[:],
                                                rhs=msgs[rr][:, base + c, :],
                                                start=(done == 0),
                                                stop=(done == nch - 1))
                                    ch_col += 1
                                    done += 1
                            epilogue(tt, ps)
                    return ch_col

                # ---- layer 1 ----
                tab2 = bigp.tile([128, T * DH], f32)

                def epi1(tt, ps):
                    src = ps[:] if ps is not None else b1_s[:]
                    u = workp.tile([128, DH], f32, tag="u")
                    nc.vector.tensor_scalar_mul(u[:], src, dinv[:, tt:tt + 1])
                    v = workp.tile([128, DH], f32, tag="v")
                    nc.vector.tensor_tensor(out=v[:], in0=u[:], in1=b1_s[:],
                                            op=mybir.AluOpType.add)
                    w = workp.tile([128, DH], f32, tag="w")
                    nc.vector.tensor_scalar_max(w[:], v[:], 0.0)
                    nc.vector.tensor_scalar_mul(tab2[:, tt * DH:(tt + 1) * DH],
                                                w[:], dinv[:, tt:tt + 1])

                aggregate(tab1_full, epi1)

                ag2_in = dramp.tile([SHP, DH], f32)
                nc.sync.dma_start(
                    out=ag2_in[:].rearrange("(p t) f -> p (t f)", p=128),
                    in_=tab2[:])
                tab2_full = dramp.tile([NTAB, DH], f32, addr_space="Shared")
                nc.gpsimd.collective_compute(
                    "AllGather", mybir.AluOpType.bypass,
                    replica_groups=[list(range(NCORE))],
                    ins=[ag2_in[:]], outs=[tab2_full[:]])

                # ---- layer 2 ----
                ybuf = bigp.tile([128, T * DOUT], f32)

                def epi2(tt, ps):
                    src = ps[:] if ps is not None else b1_s[:]
                    s1 = workp.tile([128, DH], f32, tag="s1")
                    nc.vector.tensor_scalar_mul(s1[:], src, dinv[:, tt:tt + 1])
                    tr_ps = ps2p.tile([DH, 128], f32, tag="tr")
                    nc.tensor.transpose(out=tr_ps[:], in_=s1[:], identity=ident[:])
                    tr_s = workp.tile([DH, 128], f32, tag="trs")
                    nc.vector.tensor_copy(out=tr_s[:], in_=tr_ps[:])
                    o_ps = ps2p.tile([128, DOUT], f32, tag="o")
                    nc.tensor.matmul(o_ps[:], lhsT=tr_s[:], rhs=w2_s[:],
                                     start=True, stop=True)
                    o1 = workp.tile([128, DOUT], f32, tag="o1")
                    nc.vector.tensor_tensor(out=o1[:], in0=o_ps[:], in1=b2_s[:],
                                            op=mybir.AluOpType.add)
                    nc.scalar.activation(ybuf[:, tt * DOUT:(tt + 1) * DOUT], o1[:],
                                         mybir.ActivationFunctionType.Sigmoid)

                aggregate(tab2_full, epi2)
                nc.sync.dma_start(out=y_d[:], in_=ybuf[:])

            for _pass in range(npass):
                one_pass()

    nc.compile()
    return nc


def _make_runner(nc, n_cores):
    import jax
    from jax.sharding import Mesh, NamedSharding, PartitionSpec
    from jax.experimental.shard_map import shard_map
    import concourse.mybir as mybir
    from concourse import bass2jax

    bass2jax.install_neuronx_cc_hook()
    partition_name = (nc.partition_id_tensor.name
                      if nc.partition_id_tensor else None)
    in_names, out_names, out_avals, zero_outs = [], [], [], []
    for alloc in nc.m.functions[0].allocations:
        if not isinstance(alloc, mybir.MemoryLocationSet):
            continue
        name = alloc.memorylocations[0].name
        if alloc.kind == "ExternalInput":
            if name != partition_name:
                in_names.append(name)
        elif alloc.kind == "ExternalOutput":
            out_names.append(name)
            shape = tuple(alloc.tensor_shape)
            dtype = mybir.dt.np(alloc.dtype)
            out_avals.append(jax.core.ShapedArray(shape, dtype))
            zero_outs.append(np.zeros(shape, dtype))
    n_params = len(in_names)
    all_in = list(in_names) + list(out_names)
    if partition_name is not None:
        all_in.append(partition_name)

    def _body(*args):
        operands = list(args)
        if partition_name is not None:
            operands.append(bass2jax.partition_id_tensor())
        outs = bass2jax._bass_exec_p.bind(
            *operands, out_avals=tuple(out_avals), in_names=tuple(all_in),
            out_names=tuple(out_names), lowering_input_output_aliases=(),
            sim_require_finite=True, sim_require_nnan=True, nc=nc)
        return tuple(outs)

    devices = jax.devices()[:n_cores]
    mesh = Mesh(np.asarray(devices), ("core",))
    nspec = (PartitionSpec("core"),)
    sharded = jax.jit(
        shard_map(_body, mesh=mesh, in_specs=nspec * (n_params + len(out_names)),
                  out_specs=nspec * len(out_names), check_rep=False),
        keep_unused=True)
    sh = NamedSharding(mesh, PartitionSpec("core"))

    def place(in_maps):
        per_core = [[np.asarray(m[nm]) for nm in in_names] for m in in_maps]
        concat = [np.concatenate([per_core[c][i] for c in range(n_cores)], 0)
                  for i in range(n_params)]
        concat += [np.zeros((n_cores * z.shape[0], *z.shape[1:]), z.dtype)
                   for z in zero_outs]
        placed = [jax.device_put(a, sh) for a in concat]
        jax.block_until_ready(placed)
        return placed

    def run(placed):
        out = sharded(*placed)
        jax.block_until_ready(out)
        return out

    return place, run, out_names, out_avals


def _get_compiled(edge_index_key, edge_index):
    if edge_index_key in _CACHE:
        return _CACHE[edge_index_key]
    meta, percore = _host_prep(edge_index)
    meta["idx_lens"] = [percore[0]["idx"][rr].shape[1] * 16
                        for rr in range(NRANGE)]
    nc = _build_nc(meta)
    place, run, out_names, out_avals = _make_runner(nc, NCORE)
    _CACHE[edge_index_key] = (meta, percore, place, run, out_names, out_avals)
    return _CACHE[edge_index_key]


def _in_maps(percore, x, W1, b1, W2, b2):
    x = np.asarray(x, np.float32)
    maps = []
    iota = np.tile(np.arange(128, dtype=np.float32)[None, :], (128, 1))
    for kk in range(NCORE):
        xs = np.zeros((SHP, DIN), np.float32)
        xs[:SH] = x[kk * SH:(kk + 1) * SH]
        pc = percore[kk]
        m = {
            "x_pm": np.ascontiguousarray(xs.reshape(128, T * DIN)),
            "rs": pc["rs"], "re": pc["re"],
            "w1": np.asarray(W1, np.float32),
            "w2": np.asarray(W2, np.float32),
            "b1b": np.tile(np.asarray(b1, np.float32)[None, :], (128, 1)),
            "b2b": np.tile(np.asarray(b2, np.float32)[None, :], (128, 1)),
            "iota2d": iota,
            "dl": pc["dl"],
            "dln": pc["dln"],
        }
        for rr in range(NRANGE):
            m[f"ix{rr}"] = pc["idx"][rr]
        maps.append(m)
    return maps


def kernel(x, edge_index, W1, b1, W2, b2):
    ei = np.asarray(edge_index)
    key = hash(ei.tobytes())
    meta, percore, place, run, out_names, out_avals = _get_compiled(key, ei)
    maps = _in_maps(percore, x, W1, b1, W2, b2)
    placed = place(maps)
    out = run(placed)
    yi = out_names.index("y_pm")
    y_all = np.asarray(out[yi]).reshape(NCORE, 128, T * DOUT)
    res = np.empty((N, DOUT), np.float32)
    for kk in range(NCORE):
        shard = y_all[kk].reshape(128 * T, DOUT)  # row j = p*T+t? no: p,(t f)
        # y_pm[p, t*DOUT:(t+1)*DOUT] is node j = p*T + t
        res[kk * SH:(kk + 1) * SH] = shard[:SH]
    return res



# revision 10
# speedup vs baseline: 16.1195x; 16.1195x over previous
"""2-layer GCN on 8 Trainium2 NeuronCores (Bass/Tile), self-contained.

Sharding: nodes partitioned across 8 cores (12500 rows each), weights
replicated. Per core, per layer: compute its table shard (x @ W) * dinv in
bf16, AllGather the compact [N,64] table, then aggregate incoming edge
messages with dma_gather (256B padded rows) + one-hot segment matmuls.

v3 vs v1: bf16 tables/messages/matmuls/one-hots (half HBM bytes, 4x PE),
int32->int16 window regions aligned to one AllGather, 80 large gather
calls per layer instead of 392 small ones, idx loaded once as one const,
host-precomputed xT/dinv (no device transposes in table build), fused
epilogues, AllGather writes strided into the padded gather table.
"""
import math

import numpy as np

N = 100000
E_RAW = 1600000
DIN = 64
DH = 64
DOUT = 16
NCORE = 8
SH = 12500            # nodes per core
T = 98                # dst tiles per core (128 nodes each)
SHP = 128 * T         # padded shard rows = 12544
NTAB = NCORE * SHP    # gather table rows = 100352
RSIZE = 32768         # int16 index window (rows)
NR = 4                # ceil(NTAB / RSIZE) regions
SG = 5                # tiles per gather call-group
PADP = 999.0          # dl marker for padded edges (kills one-hot col)

_CACHE = {}


def _host_prep(edge_index):
    """Build per-core gather/one-hot metadata. Returns (meta, percore)."""
    src = np.concatenate([edge_index[0], np.arange(N, dtype=np.int64)])
    dst = np.concatenate([edge_index[1], np.arange(N, dtype=np.int64)])
    src = src.astype(np.int64)
    dst = dst.astype(np.int64)

    deg = np.bincount(dst, minlength=N).astype(np.float64)
    dinv_g = (1.0 / np.sqrt(deg)).astype(np.float32)   # deg >= 1 (self loops)

    k = dst // SH                       # owning core of each edge (by dst)
    jj = dst - k * SH                   # local dst node
    t_dst = jj // 128                   # dst tile
    p_dst = jj % 128                    # psum slot
    trow = (src // SH) * SHP + (src % SH)    # gather-table row of src
    r = trow // RSIZE                   # int16 window region
    loc = trow % RSIZE

    gid = ((k * T + t_dst) * NR + r).astype(np.int64)
    order = np.argsort(gid, kind="stable")
    gid_s = gid[order]
    loc_s = loc[order].astype(np.int32)
    p_s = p_dst[order].astype(np.float32)

    ngroups = NCORE * T * NR
    counts = np.bincount(gid_s, minlength=ngroups).reshape(NCORE, T, NR)
    starts = np.zeros(ngroups + 1, np.int64)
    np.cumsum(counts.reshape(-1), out=starts[1:])

    n_chunk = -(-counts.max(axis=0) // 128)            # [T, NR]
    sbs = [list(range(s, min(s + SG, T))) for s in range(0, T, SG)]

    meta = dict(n_chunk=n_chunk, sbs=sbs)
    tot_cols = int(n_chunk.sum())
    meta["tot_cols"] = tot_cols

    import ml_dtypes
    bf = ml_dtypes.bfloat16

    percore = []
    for kk in range(NCORE):
        # idx in call order (g, r, t, c); dl in mm order (g, t, r, c)
        idx_parts = []
        dl_cols = []

        def group_arrays(tt, rr):
            g = (kk * T + tt) * NR + rr
            cnt = counts[kk, tt, rr]
            npad = n_chunk[tt, rr] * 128
            lo = np.zeros(npad, np.int32)
            pp = np.full(npad, PADP, np.float32)
            lo[:cnt] = loc_s[starts[g]:starts[g] + cnt]
            pp[:cnt] = p_s[starts[g]:starts[g] + cnt]
            return lo, pp

        for tiles in sbs:
            for rr in range(NR):
                for tt in tiles:
                    idx_parts.append(group_arrays(tt, rr)[0])
            for tt in tiles:
                for rr in range(NR):
                    pp = group_arrays(tt, rr)[1]
                    if pp.size:
                        dl_cols.append(pp.reshape(-1, 128).T)  # [128, nchunk]
        flat = np.concatenate(idx_parts).astype(np.int16)
        ix = np.tile(flat.reshape(-1, 16).T, (8, 1)).copy()     # [128, n/16]
        dl = np.concatenate(dl_cols, axis=1).astype(np.float32)  # [128, totc]

        dv = np.zeros(SHP, np.float32)
        dv[:SH] = dinv_g[kk * SH:(kk + 1) * SH]
        dinv_pm = np.ascontiguousarray(dv.reshape(T, 128).T)    # [128, T]
        percore.append(dict(ix=ix, dl=dl, dinv=dinv_pm))
    meta["idx_len"] = percore[0]["ix"].shape[1]
    return meta, percore


def _build_nc(meta, npass=1, msg_bufs=8, oh_bufs=6, mode="full",
              strided_ag=False, act_mod=0, ps_bufs=3):
    import concourse.bacc as bacc
    import concourse.mybir as mybir
    from concourse.masks import make_identity
    from concourse.tile import TileContext

    f32 = mybir.dt.float32
    bf16 = mybir.dt.bfloat16
    n_chunk = meta["n_chunk"]
    sbs = meta["sbs"]
    tot_cols = meta["tot_cols"]
    idx_len = meta["idx_len"]

    nc = bacc.Bacc("TRN2", target_bir_lowering=False, debug=False,
                   num_devices=NCORE, num_swdge_queues=4)
    xT_d = nc.dram_tensor("xT", [DIN, SHP], bf16, kind="ExternalInput")
    dinv_d = nc.dram_tensor("dinv", [128, T], f32, kind="ExternalInput")
    w1_d = nc.dram_tensor("w1", [DIN, DH], bf16, kind="ExternalInput")
    w2_d = nc.dram_tensor("w2", [DH, DOUT], bf16, kind="ExternalInput")
    b1_d = nc.dram_tensor("b1b", [128, DH], f32, kind="ExternalInput")
    b2_d = nc.dram_tensor("b2b", [128, DOUT], f32, kind="ExternalInput")
    io_d = nc.dram_tensor("iota2d", [128, 128], bf16, kind="ExternalInput")
    dl_d = nc.dram_tensor("dl", [128, tot_cols], f32, kind="ExternalInput")
    ix_d = nc.dram_tensor("ix", [128, idx_len], mybir.dt.int16,
                          kind="ExternalInput")
    y_d = nc.dram_tensor("y_pm", [128, T * DOUT], f32, kind="ExternalOutput")

    qrot = [0]

    def nextq():
        qrot[0] = (qrot[0] + 1) % 4
        return qrot[0]

    with TileContext(nc) as tc:
        with (
            tc.tile_pool(name="const", bufs=1) as constp,
            tc.tile_pool(name="big", bufs=1) as bigp,
            tc.tile_pool(name="msg", bufs=msg_bufs) as msgp,
            tc.tile_pool(name="ohp", bufs=oh_bufs) as ohp,
            tc.tile_pool(name="work", bufs=3) as workp,
            tc.tile_pool(name="ps", bufs=ps_bufs, space="PSUM") as psp,
            tc.tile_pool(name="ps2", bufs=2, space="PSUM") as ps2p,
            tc.tile_pool(name="dram", bufs=1, space="DRAM") as dramp,
        ):
            ident = constp.tile([128, 128], bf16)
            make_identity(nc, ident[:])
            w1_s = constp.tile([DIN, DH], bf16)
            nc.sync.dma_start(out=w1_s[:], in_=w1_d[:])
            w2_s = constp.tile([DH, DOUT], bf16)
            nc.sync.dma_start(out=w2_s[:], in_=w2_d[:])
            b1_s = constp.tile([128, DH], f32)
            nc.sync.dma_start(out=b1_s[:], in_=b1_d[:])
            b2_s = constp.tile([128, DOUT], f32)
            nc.sync.dma_start(out=b2_s[:], in_=b2_d[:])
            iota_s = constp.tile([128, 128], bf16)
            nc.sync.dma_start(out=iota_s[:], in_=io_d[:])
            dl_s = constp.tile([128, tot_cols], f32)
            nc.sync.dma_start(out=dl_s[:], in_=dl_d[:])
            ix_s = constp.tile([128, idx_len], mybir.dt.int16)
            nc.sync.dma_start(out=ix_s[:], in_=ix_d[:])
            dinv_s = constp.tile([128, T], f32)
            nc.sync.dma_start(out=dinv_s[:], in_=dinv_d[:])
            xT_s = constp.tile([DIN, SHP], bf16)
            nc.sync.dma_start(out=xT_s[:], in_=xT_d[:])

            def one_pass():
                # ---- table1 = (x @ W1) * dinv ----
                tab1 = bigp.tile([128, T * DH], bf16, tag="tab1")
                for t in range(T):
                    h_ps = psp.tile([128, DH], f32, tag="agg", name="h_ps")
                    nc.tensor.matmul(h_ps[:],
                                     lhsT=xT_s[:, t * 128:(t + 1) * 128],
                                     rhs=w1_s[:], start=True, stop=True)
                    nc.vector.tensor_scalar(
                        out=tab1[:, t * DH:(t + 1) * DH], in0=h_ps[:],
                        scalar1=dinv_s[:, t:t + 1], scalar2=None,
                        op0=mybir.AluOpType.mult)

                def allgather(tab_sb, tag):
                    ag_in = dramp.tile([SHP, DH], bf16, tag=f"agin{tag}")
                    nc.sync.dma_start(
                        out=ag_in[:].rearrange("(t p) f -> p t f", p=128),
                        in_=tab_sb[:].rearrange("p (t f) -> p t f", f=DH))
                    pad = dramp.tile([NTAB, 128], bf16, tag=f"pad{tag}",
                                     addr_space="Shared" if strided_ag
                                     else "Local")
                    if strided_ag:
                        nc.gpsimd.collective_compute(
                            "AllGather", mybir.AluOpType.bypass,
                            replica_groups=[list(range(NCORE))],
                            ins=[ag_in[:]], outs=[pad[:, 0:DH]])
                    else:
                        full = dramp.tile([NTAB, DH], bf16, tag=f"full{tag}",
                                          addr_space="Shared")
                        nc.gpsimd.collective_compute(
                            "AllGather", mybir.AluOpType.bypass,
                            replica_groups=[list(range(NCORE))],
                            ins=[ag_in[:]], outs=[full[:]])
                        for rr in range(NR):
                            rlen = min(RSIZE, NTAB - rr * RSIZE)
                            nc.sync.dma_start(
                                out=pad[rr * RSIZE:rr * RSIZE + rlen, 0:DH],
                                in_=full[rr * RSIZE:rr * RSIZE + rlen, :])
                    return pad

                tab1_pad = allgather(tab1, "1")

                def aggregate(pad, epilogue):
                    """Gather + one-hot segment matmuls; epilogue(t, psum)."""
                    ch_col = 0          # global dl column
                    ix_off = 0          # int16 wrapped-col offset into ix_s
                    for tiles in sbs:
                        msgs = {}
                        for rr in range(NR):
                            cols = int(sum(n_chunk[tt, rr] for tt in tiles))
                            if cols == 0 or mode == "none":
                                msgs[rr] = None
                                continue
                            m = msgp.tile([128, cols, 128], bf16, tag="m")
                            nidx = cols * 128
                            rlen = min(RSIZE, NTAB - rr * RSIZE)
                            nc.gpsimd.dma_gather(
                                out_ap=m[:],
                                in_ap=pad[rr * RSIZE:rr * RSIZE + rlen, :],
                                idxs_ap=ix_s[:, ix_off:ix_off + nidx // 16],
                                num_idxs=nidx,
                                num_idxs_reg=nidx,
                                elem_size=128,
                                queue_num=nextq(),
                                single_packet=False,
                            )
                            ix_off += nidx // 16
                            msgs[rr] = m
                        for ti, tt in enumerate(tiles):
                            nch = int(sum(n_chunk[tt, rr] for rr in range(NR)))
                            if mode == "full" and nch > 0:
                                ps = psp.tile([128, DH], f32, tag="agg",
                                              name="ps_agg")
                            else:
                                ps = None
                            done = 0
                            for rr in range(NR):
                                base = int(sum(n_chunk[t2, rr]
                                               for t2 in tiles[:ti]))
                                for c in range(int(n_chunk[tt, rr])):
                                    if mode == "full":
                                        oh = ohp.tile([128, 128], bf16,
                                                      tag="oh")
                                        if act_mod and ch_col % act_mod == 0:
                                            sq = ohp.tile([128, 128], bf16,
                                                          tag="sq")
                                            nc.scalar.activation(
                                                sq[:], iota_s[:],
                                                mybir.ActivationFunctionType.Square,
                                                bias=dl_s[:, ch_col:ch_col + 1],
                                                scale=-1.0)
                                            nc.scalar.activation(
                                                oh[:], sq[:],
                                                mybir.ActivationFunctionType.Relu,
                                                bias=1.0, scale=-1.0)
                                        else:
                                            nc.vector.tensor_scalar(
                                                out=oh[:], in0=iota_s[:],
                                                scalar1=dl_s[:, ch_col:ch_col + 1],
                                                scalar2=None,
                                                op0=mybir.AluOpType.is_equal)
                                        nc.tensor.matmul(
                                            ps[:], lhsT=oh[:],
                                            rhs=msgs[rr][:, base + c, 0:DH],
                                            start=(done == 0),
                                            stop=(done == nch - 1))
                                    ch_col += 1
                                    done += 1
                            epilogue(tt, ps)

                # ---- layer 1 ----
                tab2 = bigp.tile([128, T * DH], bf16, tag="tab2")

                def epi1(tt, ps):
                    src = ps[:] if ps is not None else b1_s[:]
                    u = workp.tile([128, DH], f32, tag="u")
                    nc.vector.tensor_scalar(
                        out=u[:], in0=src, scalar1=dinv_s[:, tt:tt + 1],
                        scalar2=None, op0=mybir.AluOpType.mult)
                    v = workp.tile([128, DH], f32, tag="v")
                    nc.vector.tensor_tensor(out=v[:], in0=u[:], in1=b1_s[:],
                                            op=mybir.AluOpType.add)
                    nc.vector.tensor_scalar(
                        out=tab2[:, tt * DH:(tt + 1) * DH], in0=v[:],
                        scalar1=dinv_s[:, tt:tt + 1], scalar2=0.0,
                        op0=mybir.AluOpType.mult, op1=mybir.AluOpType.max)

                aggregate(tab1_pad, epi1)
                tab2_pad = allgather(tab2, "2")

                # ---- layer 2 ----
                ybuf = bigp.tile([128, T * DOUT], f32, tag="ybuf")

                def epi2(tt, ps):
                    src = ps[:] if ps is not None else b1_s[:]
                    s1 = workp.tile([128, DH], bf16, tag="s1")
                    nc.vector.tensor_scalar(
                        out=s1[:], in0=src, scalar1=dinv_s[:, tt:tt + 1],
                        scalar2=None, op0=mybir.AluOpType.mult)
                    tr_ps = ps2p.tile([DH, 128], bf16, tag="tr")
                    nc.tensor.transpose(out=tr_ps[:], in_=s1[:],
                                        identity=ident[:])
                    sT = workp.tile([DH, 128], bf16, tag="sT")
                    nc.vector.tensor_copy(out=sT[:], in_=tr_ps[:])
                    o_ps = ps2p.tile([128, DOUT], f32, tag="o")
                    nc.tensor.matmul(o_ps[:], lhsT=sT[:], rhs=w2_s[:],
                                     start=True, stop=True)
                    o1 = workp.tile([128, DOUT], f32, tag="o1")
                    nc.vector.tensor_tensor(out=o1[:], in0=o_ps[:],
                                            in1=b2_s[:],
                                            op=mybir.AluOpType.add)
                    nc.scalar.activation(ybuf[:, tt * DOUT:(tt + 1) * DOUT],
                                         o1[:],
                                         mybir.ActivationFunctionType.Sigmoid)

                aggregate(tab2_pad, epi2)
                nc.sync.dma_start(out=y_d[:], in_=ybuf[:])

            for _pass in range(npass):
                one_pass()

    nc.compile()
    return nc


def _make_runner(nc, n_cores):
    import jax
    from jax.sharding import Mesh, NamedSharding, PartitionSpec
    from jax.experimental.shard_map import shard_map
    import concourse.mybir as mybir
    from concourse import bass2jax

    bass2jax.install_neuronx_cc_hook()
    partition_name = (nc.partition_id_tensor.name
                      if nc.partition_id_tensor else None)
    in_names, out_names, out_avals, zero_outs = [], [], [], []
    for alloc in nc.m.functions[0].allocations:
        if not isinstance(alloc, mybir.MemoryLocationSet):
            continue
        name = alloc.memorylocations[0].name
        if alloc.kind == "ExternalInput":
            if name != partition_name:
                in_names.append(name)
        elif alloc.kind == "ExternalOutput":
            out_names.append(name)
            shape = tuple(alloc.tensor_shape)
            dtype = mybir.dt.np(alloc.dtype)
            out_avals.append(jax.core.ShapedArray(shape, dtype))
            zero_outs.append(np.zeros(shape, dtype))
    n_params = len(in_names)
    all_in = list(in_names) + list(out_names)
    if partition_name is not None:
        all_in.append(partition_name)

    def _body(*args):
        operands = list(args)
        if partition_name is not None:
            operands.append(bass2jax.partition_id_tensor())
        outs = bass2jax._bass_exec_p.bind(
            *operands, out_avals=tuple(out_avals), in_names=tuple(all_in),
            out_names=tuple(out_names), lowering_input_output_aliases=(),
            sim_require_finite=True, sim_require_nnan=True, nc=nc)
        return tuple(outs)

    devices = jax.devices()[:n_cores]
    mesh = Mesh(np.asarray(devices), ("core",))
    nspec = (PartitionSpec("core"),)
    sharded = jax.jit(
        shard_map(_body, mesh=mesh, in_specs=nspec * (n_params + len(out_names)),
                  out_specs=nspec * len(out_names), check_rep=False),
        keep_unused=True)
    sh = NamedSharding(mesh, PartitionSpec("core"))

    def place(in_maps):
        per_core = [[np.asarray(m[nm]) for nm in in_names] for m in in_maps]
        concat = [np.concatenate([per_core[c][i] for c in range(n_cores)], 0)
                  for i in range(n_params)]
        concat += [np.zeros((n_cores * z.shape[0], *z.shape[1:]), z.dtype)
                   for z in zero_outs]
        placed = [jax.device_put(a, sh) for a in concat]
        jax.block_until_ready(placed)
        return placed

    def run(placed):
        out = sharded(*placed)
        jax.block_until_ready(out)
        return out

    return place, run, out_names, out_avals


def _get_compiled(edge_index_key, edge_index):
    if edge_index_key in _CACHE:
        return _CACHE[edge_index_key]
    meta, percore = _host_prep(edge_index)
    nc = _build_nc(meta)
    place, run, out_names, out_avals = _make_runner(nc, NCORE)
    _CACHE[edge_index_key] = (meta, percore, place, run, out_names, out_avals)
    return _CACHE[edge_index_key]


def _in_maps(percore, x, W1, b1, W2, b2):
    import ml_dtypes
    bf = ml_dtypes.bfloat16
    x = np.asarray(x, np.float32)
    maps = []
    iota = np.tile(np.arange(128, dtype=np.float32)[None, :],
                   (128, 1)).astype(bf)
    w1 = np.asarray(W1, np.float32).astype(bf)
    w2 = np.asarray(W2, np.float32).astype(bf)
    b1b = np.tile(np.asarray(b1, np.float32)[None, :], (128, 1))
    b2b = np.tile(np.asarray(b2, np.float32)[None, :], (128, 1))
    for kk in range(NCORE):
        xs = np.zeros((SHP, DIN), np.float32)
        xs[:SH] = x[kk * SH:(kk + 1) * SH]
        pc = percore[kk]
        m = {
            "xT": np.ascontiguousarray(xs.T).astype(bf),
            "dinv": pc["dinv"],
            "w1": w1, "w2": w2, "b1b": b1b, "b2b": b2b,
            "iota2d": iota,
            "dl": pc["dl"],
            "ix": pc["ix"],
        }
        maps.append(m)
    return maps


def kernel(x, edge_index, W1, b1, W2, b2):
    ei = np.asarray(edge_index)
    key = hash(ei.tobytes())
    meta, percore, place, run, out_names, out_avals = _get_compiled(key, ei)
    maps = _in_maps(percore, x, W1, b1, W2, b2)
    placed = place(maps)
    out = run(placed)
    yi = out_names.index("y_pm")
    y_all = np.asarray(out[yi]).reshape(NCORE, 128, T * DOUT)
    res = np.empty((N, DOUT), np.float32)
    for kk in range(NCORE):
        shard = y_all[kk].reshape(128, T, DOUT).transpose(1, 0, 2)
        res[kk * SH:(kk + 1) * SH] = shard.reshape(SHP, DOUT)[:SH]
    return res


# revision 11
# speedup vs baseline: 27.7849x; 1.7237x over previous
"""2-layer GCN on 8 Trainium2 NeuronCores (Bass/Tile), self-contained.

Sharding: nodes partitioned across 8 cores (12500 rows each), weights
replicated. Per core, per layer: compute its table shard (x @ W) * dinv in
bf16, AllGather the compact [N,64] table, then aggregate incoming edge
messages with dma_gather (256B padded rows) + one-hot segment matmuls.

v3 vs v1: bf16 tables/messages/matmuls/one-hots (half HBM bytes, 4x PE),
int32->int16 window regions aligned to one AllGather, 80 large gather
calls per layer instead of 392 small ones, idx loaded once as one const,
host-precomputed xT/dinv (no device transposes in table build), fused
epilogues, AllGather writes strided into the padded gather table.
"""
import math

import numpy as np

N = 100000
E_RAW = 1600000
DIN = 64
DH = 64
DOUT = 16
NCORE = 8
SH = 12500            # nodes per core
T = 98                # dst tiles per core (128 nodes each)
SHP = 128 * T         # padded shard rows = 12544
NTAB = NCORE * SHP    # gather table rows = 100352
RSIZE = 32768         # int16 index window (rows)
NR = 4                # ceil(NTAB / RSIZE) regions
SG = 5                # tiles per gather call-group
PADP = 999.0          # dl marker for padded edges (kills one-hot col)

_CACHE = {}


def _host_prep(edge_index):
    """Build per-core gather/one-hot metadata. Returns (meta, percore)."""
    src = np.concatenate([edge_index[0], np.arange(N, dtype=np.int64)])
    dst = np.concatenate([edge_index[1], np.arange(N, dtype=np.int64)])
    src = src.astype(np.int64)
    dst = dst.astype(np.int64)

    deg = np.bincount(dst, minlength=N).astype(np.float64)
    dinv_g = (1.0 / np.sqrt(deg)).astype(np.float32)   # deg >= 1 (self loops)

    k = dst // SH                       # owning core of each edge (by dst)
    jj = dst - k * SH                   # local dst node
    t_dst = jj // 128                   # dst tile
    p_dst = jj % 128                    # psum slot
    trow = (src // SH) * SHP + (src % SH)    # gather-table row of src
    r = trow // RSIZE                   # int16 window region
    loc = trow % RSIZE

    gid = ((k * T + t_dst) * NR + r).astype(np.int64)
    order = np.argsort(gid, kind="stable")
    gid_s = gid[order]
    loc_s = loc[order].astype(np.int32)
    p_s = p_dst[order].astype(np.float32)

    ngroups = NCORE * T * NR
    counts = np.bincount(gid_s, minlength=ngroups).reshape(NCORE, T, NR)
    starts = np.zeros(ngroups + 1, np.int64)
    np.cumsum(counts.reshape(-1), out=starts[1:])

    n_chunk = -(-counts.max(axis=0) // 128)            # [T, NR]
    sbs = [list(range(s, min(s + SG, T))) for s in range(0, T, SG)]

    meta = dict(n_chunk=n_chunk, sbs=sbs)
    tot_cols = int(n_chunk.sum())
    meta["tot_cols"] = tot_cols

    import ml_dtypes
    bf = ml_dtypes.bfloat16

    percore = []
    for kk in range(NCORE):
        # idx in call order (g, r, t, c); dl in mm order (g, t, r, c)
        idx_parts = []
        dl_cols = []

        def group_arrays(tt, rr):
            g = (kk * T + tt) * NR + rr
            cnt = counts[kk, tt, rr]
            npad = n_chunk[tt, rr] * 128
            lo = np.zeros(npad, np.int32)
            pp = np.full(npad, PADP, np.float32)
            lo[:cnt] = loc_s[starts[g]:starts[g] + cnt]
            pp[:cnt] = p_s[starts[g]:starts[g] + cnt]
            return lo, pp

        for tiles in sbs:
            for rr in range(NR):
                for tt in tiles:
                    idx_parts.append(group_arrays(tt, rr)[0])
            for tt in tiles:
                for rr in range(NR):
                    pp = group_arrays(tt, rr)[1]
                    if pp.size:
                        dl_cols.append(pp.reshape(-1, 128).T)  # [128, nchunk]
        flat = np.concatenate(idx_parts).astype(np.int16)
        ix = np.tile(flat.reshape(-1, 16).T, (8, 1)).copy()     # [128, n/16]
        dl = np.concatenate(dl_cols, axis=1).astype(np.float32)  # [128, totc]

        dv = np.zeros(SHP, np.float32)
        dv[:SH] = dinv_g[kk * SH:(kk + 1) * SH]
        dinv_pm = np.ascontiguousarray(dv.reshape(T, 128).T)    # [128, T]
        percore.append(dict(ix=ix, dl=dl, dinv=dinv_pm))
    meta["idx_len"] = percore[0]["ix"].shape[1]
    return meta, percore


def _build_nc(meta, npass=1, msg_bufs=8, oh_bufs=6, mode="full",
              strided_ag=False, act_mod=0, ps_bufs=3):
    import concourse.bacc as bacc
    import concourse.mybir as mybir
    from concourse.masks import make_identity
    from concourse.tile import TileContext

    f32 = mybir.dt.float32
    bf16 = mybir.dt.bfloat16
    n_chunk = meta["n_chunk"]
    sbs = meta["sbs"]
    tot_cols = meta["tot_cols"]
    idx_len = meta["idx_len"]

    nc = bacc.Bacc("TRN2", target_bir_lowering=False, debug=False,
                   num_devices=NCORE, num_swdge_queues=4)
    xT_d = nc.dram_tensor("xT", [DIN, SHP], bf16, kind="ExternalInput")
    dinv_d = nc.dram_tensor("dinv", [128, T], f32, kind="ExternalInput")
    w1_d = nc.dram_tensor("w1", [DIN, DH], bf16, kind="ExternalInput")
    w2_d = nc.dram_tensor("w2", [DH, DOUT], bf16, kind="ExternalInput")
    b1_d = nc.dram_tensor("b1b", [128, DH], f32, kind="ExternalInput")
    b2_d = nc.dram_tensor("b2b", [128, DOUT], f32, kind="ExternalInput")
    io_d = nc.dram_tensor("iota2d", [128, 128], bf16, kind="ExternalInput")
    dl_d = nc.dram_tensor("dl", [128, tot_cols], f32, kind="ExternalInput")
    ix_d = nc.dram_tensor("ix", [128, idx_len], mybir.dt.int16,
                          kind="ExternalInput")
    y_d = nc.dram_tensor("y_pm", [128, T * DOUT], f32, kind="ExternalOutput")

    qrot = [0]

    def nextq():
        qrot[0] = (qrot[0] + 1) % 4
        return qrot[0]

    with TileContext(nc) as tc:
        with (
            tc.tile_pool(name="const", bufs=1) as constp,
            tc.tile_pool(name="big", bufs=2) as bigp,
            tc.tile_pool(name="msg", bufs=msg_bufs) as msgp,
            tc.tile_pool(name="ohp", bufs=oh_bufs) as ohp,
            tc.tile_pool(name="work", bufs=3) as workp,
            tc.tile_pool(name="ps", bufs=ps_bufs, space="PSUM") as psp,
            tc.tile_pool(name="ps2", bufs=2, space="PSUM") as ps2p,
            tc.tile_pool(name="dram", bufs=2, space="DRAM") as dramp,
        ):
            ident = constp.tile([128, 128], bf16)
            make_identity(nc, ident[:])
            w1_s = constp.tile([DIN, DH], bf16)
            nc.sync.dma_start(out=w1_s[:], in_=w1_d[:])
            w2_s = constp.tile([DH, DOUT], bf16)
            nc.sync.dma_start(out=w2_s[:], in_=w2_d[:])
            b1_s = constp.tile([128, DH], f32)
            nc.sync.dma_start(out=b1_s[:], in_=b1_d[:])
            b2_s = constp.tile([128, DOUT], f32)
            nc.sync.dma_start(out=b2_s[:], in_=b2_d[:])
            iota_s = constp.tile([128, 128], bf16)
            nc.sync.dma_start(out=iota_s[:], in_=io_d[:])
            dl_s = constp.tile([128, tot_cols], f32)
            nc.sync.dma_start(out=dl_s[:], in_=dl_d[:])
            ix_s = constp.tile([128, idx_len], mybir.dt.int16)
            nc.sync.dma_start(out=ix_s[:], in_=ix_d[:])
            dinv_s = constp.tile([128, T], f32)
            nc.sync.dma_start(out=dinv_s[:], in_=dinv_d[:])
            xT_s = constp.tile([DIN, SHP], bf16)
            nc.sync.dma_start(out=xT_s[:], in_=xT_d[:])

            def one_pass():
                # ---- table1 = (x @ W1) * dinv ----
                tab1 = bigp.tile([128, T * DH], bf16, tag="tab1")
                for t in range(T):
                    h_ps = psp.tile([128, DH], f32, tag="agg", name="h_ps")
                    nc.tensor.matmul(h_ps[:],
                                     lhsT=xT_s[:, t * 128:(t + 1) * 128],
                                     rhs=w1_s[:], start=True, stop=True)
                    nc.vector.tensor_scalar(
                        out=tab1[:, t * DH:(t + 1) * DH], in0=h_ps[:],
                        scalar1=dinv_s[:, t:t + 1], scalar2=None,
                        op0=mybir.AluOpType.mult)

                def allgather(tab_sb, tag):
                    ag_in = dramp.tile([SHP, DH], bf16, tag=f"agin{tag}")
                    nc.sync.dma_start(
                        out=ag_in[:].rearrange("(t p) f -> p t f", p=128),
                        in_=tab_sb[:].rearrange("p (t f) -> p t f", f=DH))
                    pad = dramp.tile([NTAB, 128], bf16, tag=f"pad{tag}",
                                     addr_space="Shared" if strided_ag
                                     else "Local")
                    if strided_ag:
                        nc.gpsimd.collective_compute(
                            "AllGather", mybir.AluOpType.bypass,
                            replica_groups=[list(range(NCORE))],
                            ins=[ag_in[:]], outs=[pad[:, 0:DH]])
                    else:
                        full = dramp.tile([NTAB, DH], bf16, tag=f"full{tag}",
                                          addr_space="Shared")
                        nc.gpsimd.collective_compute(
                            "AllGather", mybir.AluOpType.bypass,
                            replica_groups=[list(range(NCORE))],
                            ins=[ag_in[:]], outs=[full[:]])
                        for rr in range(NR):
                            rlen = min(RSIZE, NTAB - rr * RSIZE)
                            nc.sync.dma_start(
                                out=pad[rr * RSIZE:rr * RSIZE + rlen, 0:DH],
                                in_=full[rr * RSIZE:rr * RSIZE + rlen, :])
                    return pad

                tab1_pad = allgather(tab1, "1")

                def aggregate(pad, epilogue):
                    """Gather + one-hot segment matmuls; epilogue(t, psum)."""
                    ch_col = 0          # global dl column
                    ix_off = 0          # int16 wrapped-col offset into ix_s
                    for tiles in sbs:
                        msgs = {}
                        for rr in range(NR):
                            cols = int(sum(n_chunk[tt, rr] for tt in tiles))
                            if cols == 0 or mode == "none":
                                msgs[rr] = None
                                continue
                            m = msgp.tile([128, cols, 128], bf16, tag="m")
                            nidx = cols * 128
                            rlen = min(RSIZE, NTAB - rr * RSIZE)
                            nc.gpsimd.dma_gather(
                                out_ap=m[:],
                                in_ap=pad[rr * RSIZE:rr * RSIZE + rlen, :],
                                idxs_ap=ix_s[:, ix_off:ix_off + nidx // 16],
                                num_idxs=nidx,
                                num_idxs_reg=nidx,
                                elem_size=128,
                                queue_num=nextq(),
                                single_packet=False,
                            )
                            ix_off += nidx // 16
                            msgs[rr] = m
                        for ti, tt in enumerate(tiles):
                            nch = int(sum(n_chunk[tt, rr] for rr in range(NR)))
                            if mode == "full" and nch > 0:
                                ps = psp.tile([128, DH], f32, tag="agg",
                                              name="ps_agg")
                            else:
                                ps = None
                            done = 0
                            for rr in range(NR):
                                base = int(sum(n_chunk[t2, rr]
                                               for t2 in tiles[:ti]))
                                for c in range(int(n_chunk[tt, rr])):
                                    if mode == "full":
                                        oh = ohp.tile([128, 128], bf16,
                                                      tag="oh")
                                        if act_mod and ch_col % act_mod == 0:
                                            sq = ohp.tile([128, 128], bf16,
                                                          tag="sq")
                                            nc.scalar.activation(
                                                sq[:], iota_s[:],
                                                mybir.ActivationFunctionType.Square,
                                                bias=dl_s[:, ch_col:ch_col + 1],
                                                scale=-1.0)
                                            nc.scalar.activation(
                                                oh[:], sq[:],
                                                mybir.ActivationFunctionType.Relu,
                                                bias=1.0, scale=-1.0)
                                        else:
                                            nc.vector.tensor_scalar(
                                                out=oh[:], in0=iota_s[:],
                                                scalar1=dl_s[:, ch_col:ch_col + 1],
                                                scalar2=None,
                                                op0=mybir.AluOpType.is_equal)
                                        nc.tensor.matmul(
                                            ps[:], lhsT=oh[:],
                                            rhs=msgs[rr][:, base + c, 0:DH],
                                            start=(done == 0),
                                            stop=(done == nch - 1))
                                    ch_col += 1
                                    done += 1
                            epilogue(tt, ps)

                # ---- layer 1 ----
                tab2 = bigp.tile([128, T * DH], bf16, tag="tab2")

                def epi1(tt, ps):
                    src = ps[:] if ps is not None else b1_s[:]
                    u = workp.tile([128, DH], f32, tag="u")
                    nc.vector.tensor_scalar(
                        out=u[:], in0=src, scalar1=dinv_s[:, tt:tt + 1],
                        scalar2=None, op0=mybir.AluOpType.mult)
                    v = workp.tile([128, DH], f32, tag="v")
                    nc.vector.tensor_tensor(out=v[:], in0=u[:], in1=b1_s[:],
                                            op=mybir.AluOpType.add)
                    nc.vector.tensor_scalar(
                        out=tab2[:, tt * DH:(tt + 1) * DH], in0=v[:],
                        scalar1=dinv_s[:, tt:tt + 1], scalar2=0.0,
                        op0=mybir.AluOpType.mult, op1=mybir.AluOpType.max)

                aggregate(tab1_pad, epi1)
                tab2_pad = allgather(tab2, "2")

                # ---- layer 2 ----
                ybuf = bigp.tile([128, T * DOUT], f32, tag="ybuf")

                def epi2(tt, ps):
                    src = ps[:] if ps is not None else b1_s[:]
                    s1 = workp.tile([128, DH], bf16, tag="s1")
                    nc.vector.tensor_scalar(
                        out=s1[:], in0=src, scalar1=dinv_s[:, tt:tt + 1],
                        scalar2=None, op0=mybir.AluOpType.mult)
                    tr_ps = ps2p.tile([DH, 128], bf16, tag="tr")
                    nc.tensor.transpose(out=tr_ps[:], in_=s1[:],
                                        identity=ident[:])
                    sT = workp.tile([DH, 128], bf16, tag="sT")
                    nc.vector.tensor_copy(out=sT[:], in_=tr_ps[:])
                    o_ps = ps2p.tile([128, DOUT], f32, tag="o")
                    nc.tensor.matmul(o_ps[:], lhsT=sT[:], rhs=w2_s[:],
                                     start=True, stop=True)
                    o1 = workp.tile([128, DOUT], f32, tag="o1")
                    nc.vector.tensor_tensor(out=o1[:], in0=o_ps[:],
                                            in1=b2_s[:],
                                            op=mybir.AluOpType.add)
                    nc.scalar.activation(ybuf[:, tt * DOUT:(tt + 1) * DOUT],
                                         o1[:],
                                         mybir.ActivationFunctionType.Sigmoid)

                aggregate(tab2_pad, epi2)
                nc.sync.dma_start(out=y_d[:], in_=ybuf[:])

            for _pass in range(npass):
                one_pass()

    nc.compile()
    return nc


def _make_runner(nc, n_cores):
    import jax
    from jax.sharding import Mesh, NamedSharding, PartitionSpec
    from jax.experimental.shard_map import shard_map
    import concourse.mybir as mybir
    from concourse import bass2jax

    bass2jax.install_neuronx_cc_hook()
    partition_name = (nc.partition_id_tensor.name
                      if nc.partition_id_tensor else None)
    in_names, out_names, out_avals, zero_outs = [], [], [], []
    for alloc in nc.m.functions[0].allocations:
        if not isinstance(alloc, mybir.MemoryLocationSet):
            continue
        name = alloc.memorylocations[0].name
        if alloc.kind == "ExternalInput":
            if name != partition_name:
                in_names.append(name)
        elif alloc.kind == "ExternalOutput":
            out_names.append(name)
            shape = tuple(alloc.tensor_shape)
            dtype = mybir.dt.np(alloc.dtype)
            out_avals.append(jax.core.ShapedArray(shape, dtype))
            zero_outs.append(np.zeros(shape, dtype))
    n_params = len(in_names)
    all_in = list(in_names) + list(out_names)
    if partition_name is not None:
        all_in.append(partition_name)

    def _body(*args):
        operands = list(args)
        if partition_name is not None:
            operands.append(bass2jax.partition_id_tensor())
        outs = bass2jax._bass_exec_p.bind(
            *operands, out_avals=tuple(out_avals), in_names=tuple(all_in),
            out_names=tuple(out_names), lowering_input_output_aliases=(),
            sim_require_finite=True, sim_require_nnan=True, nc=nc)
        return tuple(outs)

    devices = jax.devices()[:n_cores]
    mesh = Mesh(np.asarray(devices), ("core",))
    nspec = (PartitionSpec("core"),)
    sharded = jax.jit(
        shard_map(_body, mesh=mesh, in_specs=nspec * (n_params + len(out_names)),
                  out_specs=nspec * len(out_names), check_rep=False),
        keep_unused=True)
    sh = NamedSharding(mesh, PartitionSpec("core"))

    def place(in_maps):
        per_core = [[np.asarray(m[nm]) for nm in in_names] for m in in_maps]
        concat = [np.concatenate([per_core[c][i] for c in range(n_cores)], 0)
                  for i in range(n_params)]
        concat += [np.zeros((n_cores * z.shape[0], *z.shape[1:]), z.dtype)
                   for z in zero_outs]
        placed = [jax.device_put(a, sh) for a in concat]
        jax.block_until_ready(placed)
        return placed

    def run(placed):
        out = sharded(*placed)
        jax.block_until_ready(out)
        return out

    return place, run, out_names, out_avals


def _get_compiled(edge_index_key, edge_index):
    if edge_index_key in _CACHE:
        return _CACHE[edge_index_key]
    meta, percore = _host_prep(edge_index)
    nc = _build_nc(meta)
    place, run, out_names, out_avals = _make_runner(nc, NCORE)
    _CACHE[edge_index_key] = (meta, percore, place, run, out_names, out_avals)
    return _CACHE[edge_index_key]


def _in_maps(percore, x, W1, b1, W2, b2):
    import ml_dtypes
    bf = ml_dtypes.bfloat16
    x = np.asarray(x, np.float32)
    maps = []
    iota = np.tile(np.arange(128, dtype=np.float32)[None, :],
                   (128, 1)).astype(bf)
    w1 = np.asarray(W1, np.float32).astype(bf)
    w2 = np.asarray(W2, np.float32).astype(bf)
    b1b = np.tile(np.asarray(b1, np.float32)[None, :], (128, 1))
    b2b = np.tile(np.asarray(b2, np.float32)[None, :], (128, 1))
    for kk in range(NCORE):
        xs = np.zeros((SHP, DIN), np.float32)
        xs[:SH] = x[kk * SH:(kk + 1) * SH]
        pc = percore[kk]
        m = {
            "xT": np.ascontiguousarray(xs.T).astype(bf),
            "dinv": pc["dinv"],
            "w1": w1, "w2": w2, "b1b": b1b, "b2b": b2b,
            "iota2d": iota,
            "dl": pc["dl"],
            "ix": pc["ix"],
        }
        maps.append(m)
    return maps


def kernel(x, edge_index, W1, b1, W2, b2):
    ei = np.asarray(edge_index)
    key = hash(ei.tobytes())
    meta, percore, place, run, out_names, out_avals = _get_compiled(key, ei)
    maps = _in_maps(percore, x, W1, b1, W2, b2)
    placed = place(maps)
    out = run(placed)
    yi = out_names.index("y_pm")
    y_all = np.asarray(out[yi]).reshape(NCORE, 128, T * DOUT)
    res = np.empty((N, DOUT), np.float32)
    for kk in range(NCORE):
        shard = y_all[kk].reshape(128, T, DOUT).transpose(1, 0, 2)
        res[kk * SH:(kk + 1) * SH] = shard.reshape(SHP, DOUT)[:SH]
    return res
